# revision 16
# baseline (speedup 1.0000x reference)
"""D2Q9 lattice-Boltzmann solver step (collision + moments + streaming) on 8
Trainium2 NeuronCores.

Sharding: the (Y, X) grid is split along Y into 8 contiguous slabs of 256
rows, one per core. All math is local per cell; the periodic-shift streaming
is applied on the host during the gather (np.roll per direction — pure data
movement), so the device writes unshifted F_post and needs no halo exchange.

Per core: 4 supertiles of [128 rows x 1024 cols]; all elementwise ops at
[128, 1024]. The q-contractions run on the TensorEngine: Esum = sum_q G via
0/1 fp32 weights in group layout, and the moments rho/ux_n/uy_n via +-I
bf16 128x128 weights against bf16 copies of F (identity matmuls accumulate
scaled tiles in PSUM; +-1 weights are exact in bf16, and moment outputs
tolerate the ~0.5% bf16 input rounding against the 2e-2 gate). The EPS
reciprocal uses the ACT spline reciprocal as seed refined by one Newton
step on the DVE (RECIPROCAL_APPROX_NR): within 2 ulp of the exact-divide
accumulator, flipping no threshold cells (min |acc-9| on the task
distribution is ~1.9e-6 = 2 ulp). d = F - Feq is kept in bf16 pair-tiles
for the collision update (F_post = Feq + (1-omega)*d); |d| for EPS is
taken from the f32 difference before the downcast.
"""
from contextlib import ExitStack

import numpy as np

# ---------------- problem constants (hardcoded per contract) ----------------
Qn, Y, X = 9, 2048, 2048
N_CORES = 8
RPC = Y // N_CORES  # 256 rows per core
XS = 1024           # supertile width
EX = [1, 0, -1, 0, 1, -1, -1, 1, 0]
EY = [0, 1, 0, -1, 1, 1, -1, -1, 0]
# G-group layout for the Esum matmuls: (row offset, nrows); 9*14+9*2 = 128 rows
GROUPS = [(14 * g, 14) for g in range(9)] + [(126, 2)]

# ---- constants replicated in f32 exactly as the jax reference computes ----
_F = np.float32
ICV32 = float(_F(1.4 - 1.0))               # f32(0.4)
C_T = ICV32 / 2.0                          # T = C_T * (E2 - uu)
K1 = float(_F(_F(1.35) * _F(0.01)))        # tau-1 = (K1/(rho T) + K0) * mask
K0 = float(_F(_F(1.35) * _F(0.5)) - _F(1.0))
INV_K1 = float(_F(1.0) / _F(K1))
C1T = float(_F(1.0) / _F(0.71))            # tauT = C1T * tmw + C0T
C0T = float(_F(0.5) + _F(_F(0.5) * _F(1.0) / _F(0.71)))
EPS_BIAS = float(_F(1e-10))

_CACHE = {}


def _esum_weights():
    """lhsT weights (10, 126, 128) f32: W[g][(q*rows+dy), 14*g+dy] = 1."""
    W = np.zeros((10, 126, 128), np.float32)
    for g, (r0, rows) in enumerate(GROUPS):
        for q in range(Qn):
            for dy in range(rows):
                W[g, q * rows + dy, r0 + dy] = 1.0
    return W


def _moment_weights():
    """(2, 128, 128) +I / -I in bf16."""
    import ml_dtypes
    WM = np.zeros((2, 128, 128), ml_dtypes.bfloat16)
    idx = np.arange(128)
    WM[0, idx, idx] = 1.0
    WM[1, idx, idx] = -1.0
    return WM


def build_program():
    import concourse.bass as bass  # noqa: F401
    import concourse.tile as tile
    from concourse import bacc, mybir
    from concourse.dve_ops import RECIPROCAL_APPROX_NR

    f32 = mybir.dt.float32
    bf16 = mybir.dt.bfloat16
    OP = mybir.AluOpType
    AF = mybir.ActivationFunctionType

    nc = bacc.Bacc("TRN2", target_bir_lowering=False, debug=False,
                   enable_asserts=False, num_devices=N_CORES)

    F_ap = nc.dram_tensor("F", [Qn, RPC, X], f32, kind="ExternalInput").ap()
    G_ap = nc.dram_tensor("G", [Qn, RPC, X], f32, kind="ExternalInput").ap()
    Feq_ap = nc.dram_tensor("Feq", [Qn, RPC, X], f32, kind="ExternalInput").ap()
    W_ap = nc.dram_tensor("W", [10, 126, 128], f32, kind="ExternalInput").ap()
    WM_ap = nc.dram_tensor("WM", [2, 128, 128], bf16, kind="ExternalInput").ap()
    out_ap = nc.dram_tensor("out", [15, RPC, X], f32, kind="ExternalOutput").ap()

    def act_recip(out, in_, bias=0.0, scale=1.0):
        """ACT spline reciprocal: out = 1/(scale*in + bias), ~1.2e-5 rel."""
        nc.scalar.add_instruction(mybir.InstActivation(
            name=nc.get_next_instruction_name(),
            func=AF.Reciprocal,
            ins=[nc.scalar.lower_ap(in_),
                 mybir.ImmediateValue(dtype=f32, value=float(bias)),
                 mybir.ImmediateValue(dtype=f32, value=float(scale)),
                 mybir.ImmediateValue(dtype=f32, value=0.0)],
            outs=[nc.scalar.lower_ap(out)],
        ))

    with tile.TileContext(nc) as tc, ExitStack() as ctx:
        pool = ctx.enter_context(tc.tile_pool(name="main", bufs=1))
        pP = ctx.enter_context(tc.tile_pool(name="pp", bufs=1, space="PSUM"))

        # stationary weights: Esum groups (f32) + moment +-I (bf16)
        warena = pool.tile([126, 10 * 128], f32, tag="W", bufs=1)
        for g in range(10):
            nc.scalar.dma_start(warena[:, g * 128:(g + 1) * 128], W_ap[g, :, :])
        Wt = [warena[:, g * 128:(g + 1) * 128] for g in range(10)]
        wmom = pool.tile([128, 2 * 128], bf16, tag="WM", bufs=1)
        for m in range(2):
            nc.scalar.dma_start(wmom[:, m * 128:(m + 1) * 128], WM_ap[m, :, :])
        Ip = wmom[:, 0:128]
        Im = wmom[:, 128:256]

        def supertile(r0, x0):
            rsl = slice(r0, r0 + 128)
            xsl = slice(x0, x0 + XS)
            Ft, Qt = {}, {}
            dpair, fpair = {}, {}
            acc = None

            def gp(dst, a, b, op):
                nc.gpsimd.tensor_tensor(dst, a, b, op)

            def vv(dst, a, b, op):
                nc.vector.tensor_tensor(dst, a, b, op)

            # ---- Esum on PE (fp32 0/1 weights, group layout) ----
            es = pP.tile([128, XS], f32, tag="es", bufs=1)
            for g, (gr0, rows) in enumerate(GROUPS):
                parts = Qn * rows
                gt = pool.tile([parts, XS], f32, tag="g", bufs=3)
                nc.sync.dma_start(
                    gt[:], G_ap[:, r0 + gr0:r0 + gr0 + rows, xsl])
                for n0 in (0, 512):
                    nc.tensor.matmul(es[:, n0:n0 + 512], Wt[g][:parts, :],
                                     gt[:parts, n0:n0 + 512],
                                     start=(g == 0), stop=(g == 9))

            # ---- moment accumulators on PE (bf16 +-I weights) ----
            rhoP = pP.tile([128, XS], f32, tag="rho", bufs=1)
            uxnP = pP.tile([128, XS], f32, tag="uxn", bufs=1)
            uynP = pP.tile([128, XS], f32, tag="uyn", bufs=1)
            XQ = [q for q in range(Qn) if EX[q] != 0]
            YQ = [q for q in range(Qn) if EY[q] != 0]

            for q in range(Qn):
                f = pool.tile([128, XS], f32, tag="f", bufs=4)
                nc.sync.dma_start(f[:], F_ap[q, rsl, xsl])
                fq = pool.tile([128, XS], f32, tag="q", bufs=12)
                nc.sync.dma_start(fq[:], Feq_ap[q, rsl, xsl])
                Ft[q], Qt[q] = f, fq
                # bf16 copy of F for the PE moment matmuls
                if q % 2 == 0:
                    fpair[q // 2] = pool.tile([128, 2 * XS], bf16, tag="f16",
                                              bufs=3, name="fpair")
                f16 = fpair[q // 2][:, (q % 2) * XS:(q % 2 + 1) * XS]
                nc.scalar.activation(f16, f[:], AF.Copy)
                for n0 in (0, 512):
                    nc.tensor.matmul(rhoP[:, n0:n0 + 512], Ip,
                                     f16[:, n0:n0 + 512],
                                     start=(q == 0), stop=(q == 8))
                if EX[q] != 0:
                    wsel = Ip if EX[q] > 0 else Im
                    for n0 in (0, 512):
                        nc.tensor.matmul(uxnP[:, n0:n0 + 512], wsel,
                                         f16[:, n0:n0 + 512],
                                         start=(q == XQ[0]), stop=(q == XQ[-1]))
                if EY[q] != 0:
                    wsel = Ip if EY[q] > 0 else Im
                    for n0 in (0, 512):
                        nc.tensor.matmul(uynP[:, n0:n0 + 512], wsel,
                                         f16[:, n0:n0 + 512],
                                         start=(q == YQ[0]), stop=(q == YQ[-1]))
                # ---- EPS chain (threshold-critical path kept in f32) ----
                d32 = pool.tile([128, XS], f32, tag="d32", bufs=2)
                gp(d32[:], f[:], fq[:], OP.subtract)
                ad = pool.tile([128, XS], f32, tag="ad", bufs=2)
                nc.scalar.activation(ad[:], d32[:], AF.Abs)
                if q % 2 == 0:
                    dpair[q // 2] = pool.tile([128, 2 * XS], bf16, tag="d",
                                              bufs=5, name="dpair")
                d16 = dpair[q // 2][:, (q % 2) * XS:(q % 2 + 1) * XS]
                nc.scalar.activation(d16, d32[:], AF.Copy)
                seed = pool.tile([128, XS], f32, tag="seed", bufs=1)
                act_recip(seed[:], fq[:], bias=EPS_BIAS)
                e = pool.tile([128, XS], f32, tag="e", bufs=1)
                nc.vector._custom_dve(RECIPROCAL_APPROX_NR, out=e[:],
                                      in0=fq[:], in1=seed[:], s0=2.0)
                if q == 0:
                    acc = pool.tile([128, XS], f32, tag="acc", bufs=2)
                    vv(acc[:], ad[:], e[:], OP.mult)
                else:
                    vv(ad[:], ad[:], e[:], OP.mult)
                    vv(acc[:], acc[:], ad[:], OP.add)

            # ---------------- per-cell fields ----------------
            rho = pool.tile([128, XS], f32, tag="rho32", bufs=1)
            nc.scalar.activation(rho[:], rhoP[:], AF.Copy)  # PSUM -> SBUF
            invr = pool.tile([128, XS], f32, tag="invr", bufs=1)
            act_recip(invr[:], rhoP[:])
            ux = pool.tile([128, XS], f32, tag="ux", bufs=1)
            vv(ux[:], uxnP[:], invr[:], OP.mult)
            uy = pool.tile([128, XS], f32, tag="uy", bufs=1)
            vv(uy[:], uynP[:], invr[:], OP.mult)
            E2 = pool.tile([128, XS], f32, tag="E2", bufs=2)
            vv(E2[:], es[:], invr[:], OP.mult)
            sqx = pool.tile([128, XS], f32, tag="sqx", bufs=1)
            nc.scalar.activation(sqx[:], ux[:], AF.Square)
            sqy = pool.tile([128, XS], f32, tag="sqy", bufs=1)
            nc.scalar.activation(sqy[:], uy[:], AF.Square)
            gp(sqx[:], sqx[:], sqy[:], OP.add)          # uu
            T = pool.tile([128, XS], f32, tag="T", bufs=1)
            vv(T[:], E2[:], sqx[:], OP.subtract)
            nc.vector.tensor_scalar(T[:], T[:], C_T, 1e-6, OP.mult, OP.max)
            nc.scalar.dma_start(out_ap[12, rsl, xsl], E2[:])
            # main field stores (redundant channels w/qx/qy rebuilt on host)
            nc.scalar.dma_start(out_ap[9, rsl, xsl], rho[:])
            nc.scalar.dma_start(out_ap[10, rsl, xsl], ux[:])
            nc.scalar.dma_start(out_ap[11, rsl, xsl], uy[:])
            nc.scalar.dma_start(out_ap[13, rsl, xsl], T[:])

            # tau / omega / omegaT:  tau-1 = (K1/(rho T) + K0) * mask
            rhoT = pool.tile([128, XS], f32, tag="invr", bufs=1)
            gp(rhoT[:], rho[:], T[:], OP.mult)
            rr = pool.tile([128, XS], f32, tag="sqy", bufs=1)
            act_recip(rr[:], rhoT[:], scale=INV_K1)
            # mask in place of acc; tmw in place of rr
            nc.vector.tensor_scalar(acc[:], acc[:], 9.0, None, OP.is_lt)
            nc.vector.scalar_tensor_tensor(rr[:], rr[:], K0, acc[:],
                                           OP.add, OP.mult)   # tau - 1
            omg = pool.tile([128, XS], f32, tag="h", bufs=1)
            act_recip(omg[:], rr[:], bias=1.0)            # 1/tau
            omgT = pool.tile([128, XS], f32, tag="st", bufs=2)
            act_recip(omgT[:], rr[:], bias=C0T, scale=C1T)
            nc.scalar.dma_start(out_ap[14, rsl, xsl], omgT[:])
            om1 = pool.tile([128, XS], bf16, tag="acc", bufs=2)
            nc.scalar.activation(om1[:], omg[:], AF.Copy, bias=1.0, scale=-1.0)

            # ---------------- F_post = Feq + (1-omega)*d ----------------
            for q in range(Qn):
                d16 = dpair[q // 2][:, (q % 2) * XS:(q % 2 + 1) * XS]
                t16 = pool.tile([128, XS], bf16, tag="t16", bufs=2)
                vv(t16[:], om1[:], d16, OP.mult)
                P = pool.tile([128, XS], f32, tag="P", bufs=2)
                gp(P[:], Qt[q][:], t16[:], OP.add)
                nc.scalar.dma_start(out_ap[q, rsl, xsl], P[:])

        for r0 in (0, 128):
            for x0 in (0, XS):
                supertile(r0, x0)

    nc.compile()
    return nc


def _get_program():
    if "nc" not in _CACHE:
        _CACHE["nc"] = build_program()
    return _CACHE["nc"]


def _in_maps(F, G, Feq):
    W = _esum_weights()
    WM = _moment_weights()
    in_maps = []
    for c in range(N_CORES):
        sl = slice(c * RPC, (c + 1) * RPC)
        in_maps.append({"F": F[:, sl, :], "G": G[:, sl, :], "Feq": Feq[:, sl, :],
                        "W": W, "WM": WM})
    return in_maps


def _gather(results):
    """Assemble the full (26, Y, X) output from per-core dev tensors.

    The device ships each independent field once; channels that are
    deterministic functions of shipped fields (the w weights from T, and
    qx/qy from rho/ux/uy/E/T) are reconstructed here, extending the
    host-side broadcast of the w channels."""
    out = np.empty((26, Y, X), np.float32)
    dev_all = np.concatenate([np.asarray(results[c]["out"])[None]
                              for c in range(N_CORES)], axis=0)  # (8, 15, 256, X)
    fp = dev_all[:, 0:9].transpose(1, 0, 2, 3).reshape(Qn, Y, X)
    for q in range(Qn):
        # streaming shift applied host-side: pure reindex (np.roll)
        out[q] = np.roll(fp[q], (-EY[q], EX[q]), axis=(0, 1))
    rho, ux, uy, E, T, omgT = (dev_all[:, 9 + i].transpose(0, 1, 2)
                               .reshape(Y, X) for i in range(6))
    one_minus_T = np.float32(1.0) - T
    out[9:13] = (one_minus_T * T * np.float32(0.5))[None]
    out[13:17] = (T * T * np.float32(0.25))[None]
    out[17] = one_minus_T * one_minus_T
    out[18] = rho
    out[19] = ux
    out[20] = uy
    E = np.float32(0.5) * E
    out[21] = E
    out[22] = T
    rhoH2 = np.float32(2.0) * rho * (E + T)
    out[23] = rhoH2 * ux
    out[24] = rhoH2 * uy
    out[25] = omgT
    return out


def kernel(F, G, Feq):
    from concourse.bass_utils import run_bass_kernel_spmd

    F = np.ascontiguousarray(np.asarray(F, np.float32))
    G = np.ascontiguousarray(np.asarray(G, np.float32))
    Feq = np.ascontiguousarray(np.asarray(Feq, np.float32))
    nc = _get_program()
    res = run_bass_kernel_spmd(nc, _in_maps(F, G, Feq),
                               core_ids=list(range(N_CORES)))
    return _gather(res.results)
